# revision 11
# baseline (speedup 1.0000x reference)
"""Causal self-attention on 8 TRN2 NeuronCores (Bass/Tile, SPMD).

Problem: B=4, T=2048, C=1024, NH=16, HS=64.
  qkv = x @ W_attn + b_attn; causal softmax attention per head; y @ W_proj + b_proj.

Sharding: core = (batch b, class xh) with b = core//2, xh = core%2.
Each core computes qkv (Q^T only for its own queries) for its whole batch,
then attention + output projection for 1024 of its batch's queries: the two
512-token blocks {0,3} (xh=0) or {1,2} (xh=1) -- paired so causal work is
balanced across cores.

SPMD uniformity trick: all 8 cores run the *same* instruction stream. The
causal-extent differences between the block classes are absorbed into data:
 - each core receives its batch's tokens in a per-core permuted (128-token
   tile granularity) order, so its own query blocks always sit at permuted
   positions [0:512] and [1024:1536], and the diagonal (needs-masking)
   k-tiles always land at fixed k-slot indices (0-3 for qb0, 8-11 for qb1);
 - acausal k-slots are killed by a per-core exp bias of -30000 (exp -> 0);
 - the 4 diagonal mask patterns are position-universal and shared.

Structure: one fully interleaved pipeline. Per head-pack p (heads 2p,2p+1):
the K^T d-tile (dt=8+p), Q^T d-tile (dt=p, own queries only), and V'
column-group matmuls write their PSUM evictions *directly* into the SBUF
tiles the attention stage reads (no HBM bounce), then the pack's attention
(row-packed S^T, exp on ScalarE, masked AV with an appended ones-column in
V' providing softmax row sums) and per-pack normalization run while the
next pack's projections occupy the TensorE.

Matmuls run in bf16 (PE 1 cycle/row; fp32r measured ~2 cycles/row on HW).
PSUM accumulation is fp32. Softmax skips max-subtraction (logits ~N(0,0.4)).
"""

import numpy as np
from contextlib import ExitStack

B, T, C = 4, 2048, 1024
NH, HS = 16, 64
P = 128
NT = T // P           # 16 k-tiles per batch
NCORES = 8
VPW = NH * (HS + 1)   # 1040: V' columns (per-head 64 V cols + ones col)

# permuted 128-token tile order per class (see module docstring)
TILE_ORDER = {
    0: [0, 1, 2, 3, 4, 5, 6, 7, 12, 13, 14, 15, 8, 9, 10, 11],
    1: [4, 5, 6, 7, 0, 1, 2, 3, 8, 9, 10, 11, 12, 13, 14, 15],
}
# k-slot extents per q-block (uniform across cores)
NK0, NK1 = 8, 16


def _build_program():
    import concourse.bacc as bacc
    import concourse.tile as tile
    from concourse import mybir
    from concourse.mybir import ActivationFunctionType as AFT

    f32 = mybir.dt.float32
    bf16 = mybir.dt.bfloat16

    nc = bacc.Bacc("TRN2", target_bir_lowering=False, debug=False,
                   num_devices=NCORES)

    xd = nc.dram_tensor("x", [T, C], bf16, kind="ExternalInput").ap()
    wqk = nc.dram_tensor("wqk", [C, 2 * C], bf16, kind="ExternalInput").ap()
    bqk = nc.dram_tensor("bqk", [P, 16], f32, kind="ExternalInput").ap()
    wvp = nc.dram_tensor("wvp", [C, VPW], bf16, kind="ExternalInput").ap()
    bvp = nc.dram_tensor("bvp", [P, VPW], f32, kind="ExternalInput").ap()
    wpj = nc.dram_tensor("wproj", [C, C], bf16, kind="ExternalInput").ap()
    bpj = nc.dram_tensor("bproj", [P, C], f32, kind="ExternalInput").ap()
    masks = nc.dram_tensor("masks", [4, P, 1024], bf16, kind="ExternalInput").ap()
    biasc = nc.dram_tensor("biasc", [P, 32], f32, kind="ExternalInput").ap()
    ident = nc.dram_tensor("ident", [P, P], bf16, kind="ExternalInput").ap()
    ones64 = nc.dram_tensor("ones64", [1, 64], bf16, kind="ExternalInput").ap()
    outd = nc.dram_tensor("out", [1024, C], f32, kind="ExternalOutput").ap()

    with tile.TileContext(nc) as tc:
        with ExitStack() as octx:
            yt_pool = octx.enter_context(tc.tile_pool(name="yt", bufs=8))
            yT = [yt_pool.tile([P, 1024], bf16, tag="yt", name=f"yT{i}")
                  for i in range(8)]

            cpool = octx.enter_context(tc.tile_pool(name="const", bufs=1))
            ident_sb = cpool.tile([P, P], bf16, tag="ident")
            nc.sync.dma_start(ident_sb[:], ident)
            ones_sb = cpool.tile([1, 64], bf16, tag="ones")
            nc.sync.dma_start(ones_sb[:], ones64)
            biasc_sb = cpool.tile([P, 32], f32, tag="biasc")
            nc.sync.dma_start(biasc_sb[:], biasc)

            with ExitStack() as ctx:
                # ---- weights + constants -------------------------------
                wq_pool = ctx.enter_context(tc.tile_pool(name="wqk", bufs=8))
                wqk_sb = [wq_pool.tile([P, 2 * C], bf16, tag="wqk", name=f"wqk{i}")
                          for i in range(8)]
                for c in range(8):
                    nc.sync.dma_start(wqk_sb[c][:], wqk[c * P:(c + 1) * P, :])
                wv_pool = ctx.enter_context(tc.tile_pool(name="wvp", bufs=8))
                wvp_sb = [wv_pool.tile([P, VPW], bf16, tag="wvp", name=f"wvp{i}")
                          for i in range(8)]
                for c in range(8):
                    nc.sync.dma_start(wvp_sb[c][:], wvp[c * P:(c + 1) * P, :])
                bq_pool = ctx.enter_context(tc.tile_pool(name="bq", bufs=1))
                bqk_sb = bq_pool.tile([P, 16], f32, tag="bqk")
                nc.sync.dma_start(bqk_sb[:], bqk)
                bvp_sb = bq_pool.tile([P, VPW], f32, tag="bvp")
                nc.sync.dma_start(bvp_sb[:], bvp)
                mpool = ctx.enter_context(tc.tile_pool(name="masks", bufs=4))
                masks_sb = [mpool.tile([P, 1024], bf16, tag="mask", name=f"mask{i}")
                            for i in range(4)]
                for i in range(4):
                    nc.sync.dma_start(masks_sb[i][:], masks[i])

                # ---- pools ---------------------------------------------
                xin = ctx.enter_context(tc.tile_pool(name="xin", bufs=6))
                xT_pool = ctx.enter_context(tc.tile_pool(name="xT", bufs=32))
                vs_pool = ctx.enter_context(tc.tile_pool(name="vs", bufs=64))
                kt_pool = ctx.enter_context(tc.tile_pool(name="ktp", bufs=2))
                qt_pool = ctx.enter_context(tc.tile_pool(name="qtp", bufs=2))
                pt_pool = ctx.enter_context(tc.tile_pool(name="pt", bufs=3))
                sm_pool = ctx.enter_context(tc.tile_pool(name="sm", bufs=3))
                # PSUM: span 2x2 banks + y' 2x1 + shared 2x1 = 8 banks
                span_p = ctx.enter_context(tc.tile_pool(name="span", bufs=2, space="PSUM"))
                yp_p = ctx.enter_context(tc.tile_pool(name="yp", bufs=2, space="PSUM"))
                sh_p = ctx.enter_context(tc.tile_pool(name="shp", bufs=2, space="PSUM"))

                # ---- x^T: transpose the whole batch up front -----------
                xT = [[None] * 8 for _ in range(4)]   # [ts][c] -> [128, 512]
                for ts in range(4):
                    xrows = []
                    for tt in range(4):
                        xr = xin.tile([P, C], bf16, tag="xin")
                        r0 = ts * 512 + tt * P
                        nc.sync.dma_start(xr[:], xd[r0:r0 + P, :])
                        xrows.append(xr)
                    for c in range(8):
                        tp = sh_p.tile([P, 512], bf16, tag="shp")
                        for tt in range(4):
                            nc.tensor.transpose(tp[:, tt * P:(tt + 1) * P],
                                                xrows[tt][:, c * P:(c + 1) * P],
                                                ident_sb[:])
                        xc = xT_pool.tile([P, 512], bf16, tag="xT",
                                          name=f"xT{ts}_{c}")
                        nc.vector.tensor_copy(xc[:], tp[:])
                        xT[ts][c] = xc

                # V' column-groups: g covers packs 2g, 2g+1 (cols 260g..260g+260)
                v_sb = [[None] * NT for _ in range(4)]

                def emit_vgroup(g):
                    n0 = 260 * g
                    for s in range(NT):          # k-slot s == token tile s
                        ts, tt = s // 4, s % 4
                        acc = sh_p.tile([P, 512], f32, tag="shp")
                        for c in range(8):
                            nc.tensor.matmul(acc[:, 0:260],
                                             xT[ts][c][:, tt * P:(tt + 1) * P],
                                             wvp_sb[c][:, n0:n0 + 260],
                                             start=(c == 0), stop=(c == 7))
                        vt = vs_pool.tile([P, 260], bf16, tag="vs",
                                          name=f"v{g}_{s}")
                        nc.vector.tensor_add(vt[:], acc[:, 0:260],
                                             bvp_sb[:, n0:n0 + 260])
                        v_sb[g][s] = vt

                # ---- main pipeline over head-packs ---------------------
                for p in range(8):
                    if p % 2 == 0:
                        emit_vgroup(p // 2)

                    # K^T rows for this pack: dt = 8+p, all 4 t-supers
                    kt = kt_pool.tile([P, T], bf16, tag="kt")
                    for ts in range(4):
                        acc = sh_p.tile([P, 512], f32, tag="shp")
                        for c in range(8):
                            nc.tensor.matmul(acc[:],
                                             wqk_sb[c][:, (8 + p) * P:(9 + p) * P],
                                             xT[ts][c][:], start=(c == 0), stop=(c == 7))
                        nc.vector.tensor_scalar_add(kt[:, ts * 512:(ts + 1) * 512],
                                                    acc[:], bqk_sb[:, 8 + p:9 + p])
                    # Q^T rows: dt = p, own t-supers 0 and 2
                    qt = qt_pool.tile([P, 1024], bf16, tag="qt")
                    for qi, ts in enumerate((0, 2)):
                        acc = sh_p.tile([P, 512], f32, tag="shp")
                        for c in range(8):
                            nc.tensor.matmul(acc[:],
                                             wqk_sb[c][:, p * P:(p + 1) * P],
                                             xT[ts][c][:], start=(c == 0), stop=(c == 7))
                        nc.vector.tensor_scalar_add(qt[:, qi * 512:(qi + 1) * 512],
                                                    acc[:], bqk_sb[:, p:p + 1])

                    g, off = p // 2, (p % 2) * 130
                    sums = sm_pool.tile([P, 512], f32, tag="sums")
                    for qb in range(2):
                        nk = NK0 if qb == 0 else NK1
                        qsl = slice(qb * 512, qb * 512 + 512)
                        y1 = yp_p.tile([HS + 1, 512], f32, tag="yp")
                        y2 = yp_p.tile([HS + 1, 512], f32, tag="yp")
                        for s in range(nk):
                            ksl = slice(s * P, (s + 1) * P)
                            span = span_p.tile([P, 1024], f32, tag="span")
                            nc.tensor.matmul(span[:, 0:512], kt[0:64, ksl],
                                             qt[0:64, qsl], start=True, stop=True)
                            nc.tensor.matmul(span[:, 512:1024], kt[64:128, ksl],
                                             qt[64:128, qsl], start=True, stop=True)
                            pt = pt_pool.tile([P, 1024], bf16, tag="pt")
                            bcol = biasc_sb[:, qb * 16 + s:qb * 16 + s + 1]
                            nc.scalar.activation(pt[:], span[:], AFT.Exp,
                                                 bias=bcol, scale=0.125)
                            mi = -1
                            if qb == 0 and s < 4:
                                mi = s
                            elif qb == 1 and 8 <= s < 12:
                                mi = s - 8
                            if mi >= 0:
                                nc.vector.tensor_mul(pt[:], pt[:], masks_sb[mi][:])
                            nc.tensor.matmul(y1[:], v_sb[g][s][:, off:off + 65],
                                             pt[:, 0:512],
                                             start=(s == 0), stop=(s == nk - 1))
                            nc.tensor.matmul(y2[:], v_sb[g][s][:, off + 65:off + 130],
                                             pt[:, 512:1024],
                                             start=(s == 0), stop=(s == nk - 1))
                        # stash raw y; collect denominators at aligned partitions
                        for hh, yy in ((0, y1), (1, y2)):
                            i = qb * 2 + hh
                            nc.vector.tensor_copy(sums[32 * i:32 * i + 1, :],
                                                  yy[64:65, :])
                            nc.vector.tensor_copy(
                                yT[p][hh * 64:(hh + 1) * 64, qsl], yy[0:64, :])

                    # per-pack normalize: one batched reciprocal, 4 bcast+mul
                    recb = sm_pool.tile([P, 512], bf16, tag="recb")
                    with nc.allow_low_precision(reason="softmax denom reciprocal"):
                        nc.vector.reciprocal(recb[:], sums[:])
                    for qb in range(2):
                        qsl = slice(qb * 512, qb * 512 + 512)
                        for hh in range(2):
                            i = qb * 2 + hh
                            rcst = sm_pool.tile([1, 512], bf16, tag="rcst")
                            nc.vector.tensor_copy(rcst[:],
                                                  recb[32 * i:32 * i + 1, :])
                            bcp = sh_p.tile([64, 512], f32, tag="shp")
                            nc.tensor.matmul(bcp[:], ones_sb[:], rcst[:],
                                             start=True, stop=True)
                            nc.vector.tensor_mul(
                                yT[p][hh * 64:(hh + 1) * 64, qsl],
                                yT[p][hh * 64:(hh + 1) * 64, qsl], bcp[:])

            # ---------------- output projection --------------------------
            with ExitStack() as ctx:
                wp_pool = ctx.enter_context(tc.tile_pool(name="wpj", bufs=8))
                wpj_sb = [wp_pool.tile([P, C], bf16, tag="wpj", name=f"wpj{i}")
                          for i in range(8)]
                for c in range(8):
                    nc.sync.dma_start(wpj_sb[c][:], wpj[c * P:(c + 1) * P, :])
                bp_pool = ctx.enter_context(tc.tile_pool(name="bpj", bufs=1))
                bpj_sb = bp_pool.tile([P, C], f32, tag="bpj")
                nc.sync.dma_start(bpj_sb[:], bpj)

                pj_p = ctx.enter_context(tc.tile_pool(name="pj", bufs=4, space="PSUM"))
                ost = ctx.enter_context(tc.tile_pool(name="ost", bufs=3))
                for tt in range(8):
                    ot = ost.tile([P, C], f32, tag="ost")
                    for co in range(2):
                        acc = pj_p.tile([P, 512], f32, tag="pj")
                        for c in range(8):
                            nc.tensor.matmul(acc[:], yT[c][:, tt * P:(tt + 1) * P],
                                             wpj_sb[c][:, co * 512:(co + 1) * 512],
                                             start=(c == 0), stop=(c == 7))
                        nc.vector.tensor_add(ot[:, co * 512:(co + 1) * 512], acc[:],
                                             bpj_sb[:, co * 512:(co + 1) * 512])
                    nc.sync.dma_start(outd[tt * P:(tt + 1) * P, :], ot[:])

    nc.compile()
    return nc


_NC_CACHE = None


def _get_program():
    global _NC_CACHE
    if _NC_CACHE is None:
        _NC_CACHE = _build_program()
    return _NC_CACHE


def _host_inputs(x, W_attn, b_attn, W_proj, b_proj):
    """Build the 8 per-core input maps."""
    import ml_dtypes
    bf = ml_dtypes.bfloat16
    x = np.asarray(x, dtype=np.float32)
    W_attn = np.asarray(W_attn, dtype=np.float32)
    b_attn = np.asarray(b_attn, dtype=np.float32)
    W_proj = np.asarray(W_proj, dtype=np.float32)
    b_proj = np.asarray(b_proj, dtype=np.float32)

    wqk = np.ascontiguousarray(W_attn[:, :2 * C]).astype(bf)
    bqk = np.empty((P, 16), np.float32)
    for dt in range(16):
        bqk[:, dt] = b_attn[dt * P:(dt + 1) * P]
    # V' weights: per head 64 V columns + one zero column (ones come via bias)
    wvp = np.zeros((C, VPW), np.float32)
    bvp_row = np.zeros(VPW, np.float32)
    for h in range(NH):
        wvp[:, h * 65:h * 65 + 64] = W_attn[:, 2 * C + h * HS:2 * C + (h + 1) * HS]
        bvp_row[h * 65:h * 65 + 64] = b_attn[2 * C + h * HS:2 * C + (h + 1) * HS]
        bvp_row[h * 65 + 64] = 1.0
    wvp = wvp.astype(bf)
    bvp = np.tile(bvp_row, (P, 1))
    bpj = np.tile(b_proj, (P, 1))
    wpj = W_proj.astype(bf)

    # universal diagonal masks: mask_i[k, q] = 1 if 128*i + k <= q (dup for 2 heads)
    msk = np.zeros((4, P, 1024), np.float32)
    kk = np.arange(P)[:, None]
    qq = np.arange(512)[None, :]
    for i in range(4):
        m = (P * i + kk <= qq).astype(np.float32)
        msk[i, :, 0:512] = m
        msk[i, :, 512:1024] = m
    msk = msk.astype(bf)

    identm = np.eye(P, dtype=np.float32).astype(bf)
    ones64 = np.ones((1, 64), np.float32).astype(bf)

    in_maps = []
    for core in range(NCORES):
        b, xh = core // 2, core % 2
        order = TILE_ORDER[xh]
        tok = np.concatenate([np.arange(t * P, (t + 1) * P) for t in order])
        xc = np.ascontiguousarray(x[b][tok]).astype(bf)
        # exp bias columns [128, 32]: col = qb*16 + slot; -30000 kills acausal slots
        bc = np.zeros((P, 32), np.float32)
        if xh == 0:
            bc[:, 4:8] = -30000.0       # qb0 slots 4-7 dead (block 0)
        else:
            bc[:, 16 + 12:16 + 16] = -30000.0   # qb1 slots 12-15 dead (block 2)
        in_maps.append({
            "x": xc, "wqk": wqk, "bqk": bqk, "wvp": wvp, "bvp": bvp,
            "wproj": wpj, "bproj": bpj, "masks": msk, "biasc": bc,
            "ident": identm, "ones64": ones64,
        })
    return in_maps


def run(inputs, trace=False, tmpdir=None):
    from concourse.bass_utils import run_bass_kernel_spmd
    nc = _get_program()
    in_maps = _host_inputs(**inputs)
    res = run_bass_kernel_spmd(nc, in_maps, core_ids=list(range(NCORES)),
                               trace=trace, tmpdir=tmpdir)
    out = np.empty((B, T, C), np.float32)
    for core in range(NCORES):
        b, xh = core // 2, core % 2
        o = res.results[core]["out"]
        blk0, blk1 = (0, 3) if xh == 0 else (1, 2)
        out[b, blk0 * 512:(blk0 + 1) * 512] = o[0:512]
        out[b, blk1 * 512:(blk1 + 1) * 512] = o[512:1024]
    return out, res


def kernel(x, W_attn, b_attn, W_proj, b_proj):
    out, _ = run(dict(x=x, W_attn=W_attn, b_attn=b_attn,
                      W_proj=W_proj, b_proj=b_proj))
    return out


# revision 15
# speedup vs baseline: 1.1515x; 1.1515x over previous
"""Causal self-attention on 8 TRN2 NeuronCores (Bass/Tile, SPMD).

Problem: B=4, T=2048, C=1024, NH=16, HS=64.
  qkv = x @ W_attn + b_attn; causal softmax attention per head; y @ W_proj + b_proj.

Sharding: core = (batch b, class xh) with b = core//2, xh = core%2.
Each core computes qkv (Q^T only for its own queries) for its whole batch,
then attention + output projection for 1024 of its batch's queries: the two
512-token blocks {0,3} (xh=0) or {1,2} (xh=1) -- paired so causal work is
balanced across cores.

SPMD uniformity trick: all 8 cores run the *same* instruction stream. The
causal-extent differences between the block classes are absorbed into data:
 - each core receives its batch's tokens in a per-core permuted (128-token
   tile granularity) order, so its own query blocks always sit at permuted
   positions [0:512] and [1024:1536], and the diagonal (needs-masking)
   k-tiles always land at fixed k-slot indices (0-3 for qb0, 8-11 for qb1);
 - acausal k-slots are killed by a per-core exp bias of -30000 (exp -> 0);
 - the 4 diagonal mask patterns are position-universal and shared.

Structure: one fully interleaved pipeline. Per head-pack p (heads 2p,2p+1):
the K^T d-tile (dt=8+p), Q^T d-tile (dt=p, own queries only), and V'
column-group matmuls write their PSUM evictions *directly* into the SBUF
tiles the attention stage reads (no HBM bounce), then the pack's attention
(row-packed S^T, exp on ScalarE, masked AV with an appended ones-column in
V' providing softmax row sums) and per-pack normalization run while the
next pack's projections occupy the TensorE.

Matmuls run in bf16 (PE 1 cycle/row; fp32r measured ~2 cycles/row on HW).
PSUM accumulation is fp32. Softmax skips max-subtraction (logits ~N(0,0.4)).
"""

import numpy as np
from contextlib import ExitStack

B, T, C = 4, 2048, 1024
NH, HS = 16, 64
P = 128
NT = T // P           # 16 k-tiles per batch
NCORES = 8
VPW = NH * (HS + 1)   # 1040: V' columns (per-head 64 V cols + ones col)

# permuted 128-token tile order per class (see module docstring)
TILE_ORDER = {
    0: [0, 1, 2, 3, 4, 5, 6, 7, 12, 13, 14, 15, 8, 9, 10, 11],
    1: [4, 5, 6, 7, 0, 1, 2, 3, 8, 9, 10, 11, 12, 13, 14, 15],
}
# k-slot extents per q-block (uniform across cores)
NK0, NK1 = 8, 16


def _build_program():
    import concourse.bacc as bacc
    import concourse.tile as tile
    from concourse import mybir
    from concourse.mybir import ActivationFunctionType as AFT

    f32 = mybir.dt.float32
    bf16 = mybir.dt.bfloat16

    nc = bacc.Bacc("TRN2", target_bir_lowering=False, debug=False,
                   num_devices=NCORES)

    xd = nc.dram_tensor("x", [T, C], bf16, kind="ExternalInput").ap()
    wqk = nc.dram_tensor("wqk", [C, 2 * C], bf16, kind="ExternalInput").ap()
    bqk = nc.dram_tensor("bqk", [P, 16], f32, kind="ExternalInput").ap()
    wvp = nc.dram_tensor("wvp", [C, VPW], bf16, kind="ExternalInput").ap()
    bvp = nc.dram_tensor("bvp", [P, VPW], f32, kind="ExternalInput").ap()
    wpj = nc.dram_tensor("wproj", [C, C], bf16, kind="ExternalInput").ap()
    bpj = nc.dram_tensor("bproj", [P, C], f32, kind="ExternalInput").ap()
    masks = nc.dram_tensor("masks", [4, P, 1024], bf16, kind="ExternalInput").ap()
    biasc = nc.dram_tensor("biasc", [P, 32], f32, kind="ExternalInput").ap()
    ident = nc.dram_tensor("ident", [P, P], bf16, kind="ExternalInput").ap()
    ones64 = nc.dram_tensor("ones64", [1, 64], bf16, kind="ExternalInput").ap()
    outd = nc.dram_tensor("out", [1024, C], f32, kind="ExternalOutput").ap()

    with tile.TileContext(nc) as tc:
        with ExitStack() as octx:
            yt_pool = octx.enter_context(tc.tile_pool(name="yt", bufs=8))
            yT = [yt_pool.tile([P, 1024], bf16, tag="yt", name=f"yT{i}")
                  for i in range(8)]

            cpool = octx.enter_context(tc.tile_pool(name="const", bufs=1))
            ident_sb = cpool.tile([P, P], bf16, tag="ident")
            nc.sync.dma_start(ident_sb[:], ident)
            ones_sb = cpool.tile([1, 64], bf16, tag="ones")
            nc.sync.dma_start(ones_sb[:], ones64)
            biasc_sb = cpool.tile([P, 32], f32, tag="biasc")
            nc.sync.dma_start(biasc_sb[:], biasc)

            with ExitStack() as ctx:
                # ---- pools ---------------------------------------------
                xin = ctx.enter_context(tc.tile_pool(name="xin", bufs=16))
                xT_pool = ctx.enter_context(tc.tile_pool(name="xT", bufs=32))
                vs_pool = ctx.enter_context(tc.tile_pool(name="vs", bufs=64))
                kt_pool = ctx.enter_context(tc.tile_pool(name="ktp", bufs=2))
                qt_pool = ctx.enter_context(tc.tile_pool(name="qtp", bufs=2))
                pt_pool = ctx.enter_context(tc.tile_pool(name="pt", bufs=3))
                sm_pool = ctx.enter_context(tc.tile_pool(name="sm", bufs=3))
                # PSUM: span 2x2 banks + y' 2x1 + shared 2x1 = 8 banks
                span_p = ctx.enter_context(tc.tile_pool(name="span", bufs=2, space="PSUM"))
                yp_p = ctx.enter_context(tc.tile_pool(name="yp", bufs=2, space="PSUM"))
                sh_p = ctx.enter_context(tc.tile_pool(name="shp", bufs=2, space="PSUM"))

                # ---- input DMAs: x rows first (transposes gate on them),
                # weights queue behind.
                xrows = []
                for g in range(NT):
                    xr = xin.tile([P, C], bf16, tag="xin", name=f"xin{g}")
                    nc.sync.dma_start(xr[:], xd[g * P:(g + 1) * P, :])
                    xrows.append(xr)

                wq_pool = ctx.enter_context(tc.tile_pool(name="wqk", bufs=8))
                wqk_sb = [wq_pool.tile([P, 2 * C], bf16, tag="wqk", name=f"wqk{i}")
                          for i in range(8)]
                for c in range(8):
                    nc.sync.dma_start(wqk_sb[c][:], wqk[c * P:(c + 1) * P, :])
                wv_pool = ctx.enter_context(tc.tile_pool(name="wvp", bufs=8))
                wvp_sb = [wv_pool.tile([P, VPW], bf16, tag="wvp", name=f"wvp{i}")
                          for i in range(8)]
                for c in range(8):
                    nc.sync.dma_start(wvp_sb[c][:], wvp[c * P:(c + 1) * P, :])
                bq_pool = ctx.enter_context(tc.tile_pool(name="bq", bufs=1))
                bqk_sb = bq_pool.tile([P, 16], f32, tag="bqk")
                nc.sync.dma_start(bqk_sb[:], bqk)
                bvp_sb = bq_pool.tile([P, VPW], f32, tag="bvp")
                nc.sync.dma_start(bvp_sb[:], bvp)
                mpool = ctx.enter_context(tc.tile_pool(name="masks", bufs=4))
                masks_sb = [mpool.tile([P, 1024], bf16, tag="mask", name=f"mask{i}")
                            for i in range(4)]
                for i in range(4):
                    nc.sync.dma_start(masks_sb[i][:], masks[i])

                # ---- x^T: transpose the whole batch up front -----------
                xT = [[None] * 8 for _ in range(4)]   # [ts][c] -> [128, 512]
                for ts in range(4):
                    for c in range(8):
                        tp = sh_p.tile([P, 512], bf16, tag="shp")
                        for tt in range(4):
                            nc.tensor.transpose(tp[:, tt * P:(tt + 1) * P],
                                                xrows[ts * 4 + tt][:, c * P:(c + 1) * P],
                                                ident_sb[:])
                        xc = xT_pool.tile([P, 512], bf16, tag="xT",
                                          name=f"xT{ts}_{c}")
                        nc.vector.tensor_copy(xc[:], tp[:])
                        xT[ts][c] = xc

                # ---- qkv emission units (software pipelining) ----------
                # Each unit emits one PSUM accumulation (8 matmuls + evict).
                v_sb = [[None] * NT for _ in range(4)]
                kt_tiles = {}
                qt_tiles = {}

                def unit_v(g, s):
                    def emit():
                        n0 = 260 * g
                        ts, tt = s // 4, s % 4
                        acc = sh_p.tile([P, 512], f32, tag="shp")
                        for c in range(8):
                            nc.tensor.matmul(acc[:, 0:260],
                                             xT[ts][c][:, tt * P:(tt + 1) * P],
                                             wvp_sb[c][:, n0:n0 + 260],
                                             start=(c == 0), stop=(c == 7))
                        vt = vs_pool.tile([P, 260], bf16, tag="vs",
                                          name=f"v{g}_{s}")
                        nc.vector.tensor_add(vt[:], acc[:, 0:260],
                                             bvp_sb[:, n0:n0 + 260])
                        v_sb[g][s] = vt
                    return emit

                def unit_k(p, ts):
                    def emit():
                        if p not in kt_tiles:
                            kt_tiles[p] = kt_pool.tile([P, T], bf16, tag="kt",
                                                       name=f"kt{p}")
                        kt = kt_tiles[p]
                        acc = sh_p.tile([P, 512], f32, tag="shp")
                        for c in range(8):
                            nc.tensor.matmul(acc[:],
                                             wqk_sb[c][:, (8 + p) * P:(9 + p) * P],
                                             xT[ts][c][:], start=(c == 0), stop=(c == 7))
                        nc.vector.tensor_scalar_add(kt[:, ts * 512:(ts + 1) * 512],
                                                    acc[:], bqk_sb[:, 8 + p:9 + p])
                    return emit

                def unit_q(p, qi):
                    def emit():
                        if p not in qt_tiles:
                            qt_tiles[p] = qt_pool.tile([P, 1024], bf16, tag="qt",
                                                       name=f"qt{p}")
                        qt = qt_tiles[p]
                        ts = (0, 2)[qi]
                        acc = sh_p.tile([P, 512], f32, tag="shp")
                        for c in range(8):
                            nc.tensor.matmul(acc[:],
                                             wqk_sb[c][:, p * P:(p + 1) * P],
                                             xT[ts][c][:], start=(c == 0), stop=(c == 7))
                        nc.vector.tensor_scalar_add(qt[:, qi * 512:(qi + 1) * 512],
                                                    acc[:], bqk_sb[:, p:p + 1])
                    return emit

                def qkv_units(p):
                    units = []
                    if p % 2 == 0:
                        units += [unit_v(p // 2, s) for s in range(NT)]
                    units += [unit_k(p, ts) for ts in range(4)]
                    units += [unit_q(p, qi) for qi in range(2)]
                    return units

                # ---- main pipeline over head-packs ---------------------
                for u in qkv_units(0):      # prologue
                    u()

                for p in range(8):
                    pend = qkv_units(p + 1) if p < 8 - 1 else []
                    total_u, emitted, si = len(pend), 0, 0
                    kt, qt = kt_tiles[p], qt_tiles[p]
                    g, off = p // 2, (p % 2) * 130
                    sums = sm_pool.tile([P, 512], f32, tag="sums")
                    for qb in range(2):
                        nk = NK0 if qb == 0 else NK1
                        qsl = slice(qb * 512, qb * 512 + 512)
                        y1 = yp_p.tile([HS + 1, 512], f32, tag="yp")
                        y2 = yp_p.tile([HS + 1, 512], f32, tag="yp")
                        for s in range(nk):
                            ksl = slice(s * P, (s + 1) * P)
                            span = span_p.tile([P, 1024], f32, tag="span")
                            nc.tensor.matmul(span[:, 0:512], kt[0:64, ksl],
                                             qt[0:64, qsl], start=True, stop=True)
                            nc.tensor.matmul(span[:, 512:1024], kt[64:128, ksl],
                                             qt[64:128, qsl], start=True, stop=True)
                            pt = pt_pool.tile([P, 1024], bf16, tag="pt")
                            bcol = biasc_sb[:, qb * 16 + s:qb * 16 + s + 1]
                            nc.scalar.activation(pt[:], span[:], AFT.Exp,
                                                 bias=bcol, scale=0.125)
                            mi = -1
                            if qb == 0 and s < 4:
                                mi = s
                            elif qb == 1 and 8 <= s < 12:
                                mi = s - 8
                            if mi >= 0:
                                nc.vector.tensor_mul(pt[:], pt[:], masks_sb[mi][:])
                            nc.tensor.matmul(y1[:], v_sb[g][s][:, off:off + 65],
                                             pt[:, 0:512],
                                             start=(s == 0), stop=(s == nk - 1))
                            nc.tensor.matmul(y2[:], v_sb[g][s][:, off + 65:off + 130],
                                             pt[:, 512:1024],
                                             start=(s == 0), stop=(s == nk - 1))
                            # spread next pack's qkv accumulations across this
                            # pack's attention slots (keeps PE fed while ACT
                            # drains the exp backlog)
                            si += 1
                            want = total_u * si // (NK0 + NK1)
                            while emitted < want:
                                pend.pop(0)()
                                emitted += 1
                        # stash raw y; collect denominators at aligned partitions
                        for hh, yy in ((0, y1), (1, y2)):
                            i = qb * 2 + hh
                            nc.vector.tensor_copy(sums[32 * i:32 * i + 1, :],
                                                  yy[64:65, :])
                            nc.vector.tensor_copy(
                                yT[p][hh * 64:(hh + 1) * 64, qsl], yy[0:64, :])

                    # per-pack normalize: one batched reciprocal, 4 bcast+mul
                    recb = sm_pool.tile([P, 512], bf16, tag="recb")
                    with nc.allow_low_precision(reason="softmax denom reciprocal"):
                        nc.vector.reciprocal(recb[:], sums[:])
                    for qb in range(2):
                        qsl = slice(qb * 512, qb * 512 + 512)
                        for hh in range(2):
                            i = qb * 2 + hh
                            rcst = sm_pool.tile([1, 512], bf16, tag="rcst")
                            nc.vector.tensor_copy(rcst[:],
                                                  recb[32 * i:32 * i + 1, :])
                            bcp = sh_p.tile([64, 512], f32, tag="shp")
                            nc.tensor.matmul(bcp[:], ones_sb[:], rcst[:],
                                             start=True, stop=True)
                            nc.vector.tensor_mul(
                                yT[p][hh * 64:(hh + 1) * 64, qsl],
                                yT[p][hh * 64:(hh + 1) * 64, qsl], bcp[:])

            # ---------------- output projection --------------------------
            with ExitStack() as ctx:
                wp_pool = ctx.enter_context(tc.tile_pool(name="wpj", bufs=8))
                wpj_sb = [wp_pool.tile([P, C], bf16, tag="wpj", name=f"wpj{i}")
                          for i in range(8)]
                for c in range(8):
                    nc.sync.dma_start(wpj_sb[c][:], wpj[c * P:(c + 1) * P, :])
                bp_pool = ctx.enter_context(tc.tile_pool(name="bpj", bufs=1))
                bpj_sb = bp_pool.tile([P, C], f32, tag="bpj")
                nc.sync.dma_start(bpj_sb[:], bpj)

                pj_p = ctx.enter_context(tc.tile_pool(name="pj", bufs=4, space="PSUM"))
                ost = ctx.enter_context(tc.tile_pool(name="ost", bufs=3))
                for tt in range(8):
                    ot = ost.tile([P, C], f32, tag="ost")
                    for co in range(2):
                        acc = pj_p.tile([P, 512], f32, tag="pj")
                        for c in range(8):
                            nc.tensor.matmul(acc[:], yT[c][:, tt * P:(tt + 1) * P],
                                             wpj_sb[c][:, co * 512:(co + 1) * 512],
                                             start=(c == 0), stop=(c == 7))
                        nc.vector.tensor_add(ot[:, co * 512:(co + 1) * 512], acc[:],
                                             bpj_sb[:, co * 512:(co + 1) * 512])
                    nc.sync.dma_start(outd[tt * P:(tt + 1) * P, :], ot[:])

    nc.compile()
    return nc


_NC_CACHE = None


def _get_program():
    global _NC_CACHE
    if _NC_CACHE is None:
        _NC_CACHE = _build_program()
    return _NC_CACHE


def _host_inputs(x, W_attn, b_attn, W_proj, b_proj):
    """Build the 8 per-core input maps."""
    import ml_dtypes
    bf = ml_dtypes.bfloat16
    x = np.asarray(x, dtype=np.float32)
    W_attn = np.asarray(W_attn, dtype=np.float32)
    b_attn = np.asarray(b_attn, dtype=np.float32)
    W_proj = np.asarray(W_proj, dtype=np.float32)
    b_proj = np.asarray(b_proj, dtype=np.float32)

    wqk = np.ascontiguousarray(W_attn[:, :2 * C]).astype(bf)
    bqk = np.empty((P, 16), np.float32)
    for dt in range(16):
        bqk[:, dt] = b_attn[dt * P:(dt + 1) * P]
    # V' weights: per head 64 V columns + one zero column (ones come via bias)
    wvp = np.zeros((C, VPW), np.float32)
    bvp_row = np.zeros(VPW, np.float32)
    for h in range(NH):
        wvp[:, h * 65:h * 65 + 64] = W_attn[:, 2 * C + h * HS:2 * C + (h + 1) * HS]
        bvp_row[h * 65:h * 65 + 64] = b_attn[2 * C + h * HS:2 * C + (h + 1) * HS]
        bvp_row[h * 65 + 64] = 1.0
    wvp = wvp.astype(bf)
    bvp = np.tile(bvp_row, (P, 1))
    bpj = np.tile(b_proj, (P, 1))
    wpj = W_proj.astype(bf)

    # universal diagonal masks: mask_i[k, q] = 1 if 128*i + k <= q (dup for 2 heads)
    msk = np.zeros((4, P, 1024), np.float32)
    kk = np.arange(P)[:, None]
    qq = np.arange(512)[None, :]
    for i in range(4):
        m = (P * i + kk <= qq).astype(np.float32)
        msk[i, :, 0:512] = m
        msk[i, :, 512:1024] = m
    msk = msk.astype(bf)

    identm = np.eye(P, dtype=np.float32).astype(bf)
    ones64 = np.ones((1, 64), np.float32).astype(bf)

    in_maps = []
    for core in range(NCORES):
        b, xh = core // 2, core % 2
        order = TILE_ORDER[xh]
        tok = np.concatenate([np.arange(t * P, (t + 1) * P) for t in order])
        xc = np.ascontiguousarray(x[b][tok]).astype(bf)
        # exp bias columns [128, 32]: col = qb*16 + slot; -30000 kills acausal slots
        bc = np.zeros((P, 32), np.float32)
        if xh == 0:
            bc[:, 4:8] = -30000.0       # qb0 slots 4-7 dead (block 0)
        else:
            bc[:, 16 + 12:16 + 16] = -30000.0   # qb1 slots 12-15 dead (block 2)
        in_maps.append({
            "x": xc, "wqk": wqk, "bqk": bqk, "wvp": wvp, "bvp": bvp,
            "wproj": wpj, "bproj": bpj, "masks": msk, "biasc": bc,
            "ident": identm, "ones64": ones64,
        })
    return in_maps


def run(inputs, trace=False, tmpdir=None):
    from concourse.bass_utils import run_bass_kernel_spmd
    nc = _get_program()
    in_maps = _host_inputs(**inputs)
    res = run_bass_kernel_spmd(nc, in_maps, core_ids=list(range(NCORES)),
                               trace=trace, tmpdir=tmpdir)
    out = np.empty((B, T, C), np.float32)
    for core in range(NCORES):
        b, xh = core // 2, core % 2
        o = res.results[core]["out"]
        blk0, blk1 = (0, 3) if xh == 0 else (1, 2)
        out[b, blk0 * 512:(blk0 + 1) * 512] = o[0:512]
        out[b, blk1 * 512:(blk1 + 1) * 512] = o[512:1024]
    return out, res


def kernel(x, W_attn, b_attn, W_proj, b_proj):
    out, _ = run(dict(x=x, W_attn=W_attn, b_attn=b_attn,
                      W_proj=W_proj, b_proj=b_proj))
    return out


# revision 18
# speedup vs baseline: 1.2512x; 1.0866x over previous
"""Causal self-attention on 8 TRN2 NeuronCores (Bass/Tile, SPMD).

Problem: B=4, T=2048, C=1024, NH=16, HS=64.
  qkv = x @ W_attn + b_attn; causal softmax attention per head; y @ W_proj + b_proj.

Sharding: core = (batch b, class xh) with b = core//2, xh = core%2.
Each core computes qkv (Q^T only for its own queries) for its whole batch,
then attention + output projection for 1024 of its batch's queries: the two
512-token blocks {0,3} (xh=0) or {1,2} (xh=1) -- paired so causal work is
balanced across cores.

SPMD uniformity trick: all 8 cores run the *same* instruction stream. The
causal-extent differences between the block classes are absorbed into data:
 - each core receives its batch's tokens in a per-core permuted (128-token
   tile granularity) order, so its own query blocks always sit at permuted
   positions [0:512] and [1024:1536], and the diagonal (needs-masking)
   k-tiles always land at fixed k-slot indices (0-3 for qb0, 8-11 for qb1);
 - acausal k-slots are killed by a per-core exp bias of -30000 (exp -> 0);
 - the 4 diagonal mask patterns are position-universal and shared.

Structure: one fully interleaved pipeline. Per head-pack p (heads 2p,2p+1):
the K^T d-tile (dt=8+p), Q^T d-tile (dt=p, own queries only), and V'
column-group matmuls write their PSUM evictions *directly* into the SBUF
tiles the attention stage reads (no HBM bounce), then the pack's attention
(row-packed S^T, exp on ScalarE, masked AV with an appended ones-column in
V' providing softmax row sums) and per-pack normalization run while the
next pack's projections occupy the TensorE.

Matmuls run in bf16 (PE 1 cycle/row; fp32r measured ~2 cycles/row on HW).
PSUM accumulation is fp32. Softmax skips max-subtraction (logits ~N(0,0.4)).
"""

import numpy as np
from contextlib import ExitStack

B, T, C = 4, 2048, 1024
NH, HS = 16, 64
P = 128
NT = T // P           # 16 k-tiles per batch
NCORES = 8
VPW = NH * (HS + 1)   # 1040: V' columns (per-head 64 V cols + ones col)

# permuted 128-token tile order per class (see module docstring)
TILE_ORDER = {
    0: [0, 1, 2, 3, 4, 5, 6, 7, 12, 13, 14, 15, 8, 9, 10, 11],
    1: [4, 5, 6, 7, 0, 1, 2, 3, 8, 9, 10, 11, 12, 13, 14, 15],
}
# k-slot extents per q-block (uniform across cores)
NK0, NK1 = 8, 16


def _build_program():
    import concourse.bacc as bacc
    import concourse.tile as tile
    from concourse import mybir
    from concourse.mybir import ActivationFunctionType as AFT

    f32 = mybir.dt.float32
    bf16 = mybir.dt.bfloat16

    nc = bacc.Bacc("TRN2", target_bir_lowering=False, debug=False,
                   num_devices=NCORES)

    xd = nc.dram_tensor("x", [T, C], bf16, kind="ExternalInput").ap()
    wqk = nc.dram_tensor("wqk", [C, 2 * C], bf16, kind="ExternalInput").ap()
    bqk = nc.dram_tensor("bqk", [P, 16], f32, kind="ExternalInput").ap()
    wvp = nc.dram_tensor("wvp", [C, VPW], bf16, kind="ExternalInput").ap()
    bvp = nc.dram_tensor("bvp", [P, VPW], f32, kind="ExternalInput").ap()
    wpj = nc.dram_tensor("wproj", [C, C], bf16, kind="ExternalInput").ap()
    bpj = nc.dram_tensor("bproj", [P, C], f32, kind="ExternalInput").ap()
    masks = nc.dram_tensor("masks", [4, P, 1024], bf16, kind="ExternalInput").ap()
    biasc = nc.dram_tensor("biasc", [P, 32], f32, kind="ExternalInput").ap()
    ident = nc.dram_tensor("ident", [P, P], bf16, kind="ExternalInput").ap()
    ones64 = nc.dram_tensor("ones64", [1, 64], bf16, kind="ExternalInput").ap()
    outd = nc.dram_tensor("out", [1024, C], f32, kind="ExternalOutput").ap()

    with tile.TileContext(nc) as tc:
        with ExitStack() as octx:
            yt_pool = octx.enter_context(tc.tile_pool(name="yt", bufs=8))
            yT = [yt_pool.tile([P, 1024], bf16, tag="yt", name=f"yT{i}")
                  for i in range(8)]

            cpool = octx.enter_context(tc.tile_pool(name="const", bufs=1))
            ident_sb = cpool.tile([P, P], bf16, tag="ident")
            nc.sync.dma_start(ident_sb[:], ident)
            ones_sb = cpool.tile([1, 64], bf16, tag="ones")
            nc.sync.dma_start(ones_sb[:], ones64)
            biasc_sb = cpool.tile([P, 32], f32, tag="biasc")
            nc.sync.dma_start(biasc_sb[:], biasc)

            with ExitStack() as ctx:
                # ---- pools ---------------------------------------------
                xin = ctx.enter_context(tc.tile_pool(name="xin", bufs=16))
                xT_pool = ctx.enter_context(tc.tile_pool(name="xT", bufs=32))
                vs_pool = ctx.enter_context(tc.tile_pool(name="vs", bufs=64))
                kt_pool = ctx.enter_context(tc.tile_pool(name="ktp", bufs=2))
                qt_pool = ctx.enter_context(tc.tile_pool(name="qtp", bufs=2))
                pt_pool = ctx.enter_context(tc.tile_pool(name="pt", bufs=3))
                sm_pool = ctx.enter_context(tc.tile_pool(name="sm", bufs=3))
                # PSUM: span 2x2 banks + y' 2x1 + shared 2x1 = 8 banks
                span_p = ctx.enter_context(tc.tile_pool(name="span", bufs=2, space="PSUM"))
                yp_p = ctx.enter_context(tc.tile_pool(name="yp", bufs=2, space="PSUM"))
                sh_p = ctx.enter_context(tc.tile_pool(name="shp", bufs=2, space="PSUM"))

                # ---- input DMAs: x rows first (transposes gate on them),
                # weights queue behind.
                xrows = []
                for g in range(NT):
                    xr = xin.tile([P, C], bf16, tag="xin", name=f"xin{g}")
                    nc.sync.dma_start(xr[:], xd[g * P:(g + 1) * P, :])
                    xrows.append(xr)

                wq_pool = ctx.enter_context(tc.tile_pool(name="wqk", bufs=8))
                wqk_sb = [wq_pool.tile([P, 2 * C], bf16, tag="wqk", name=f"wqk{i}")
                          for i in range(8)]
                for c in range(8):
                    nc.sync.dma_start(wqk_sb[c][:], wqk[c * P:(c + 1) * P, :])
                wv_pool = ctx.enter_context(tc.tile_pool(name="wvp", bufs=8))
                wvp_sb = [wv_pool.tile([P, VPW], bf16, tag="wvp", name=f"wvp{i}")
                          for i in range(8)]
                for c in range(8):
                    nc.sync.dma_start(wvp_sb[c][:], wvp[c * P:(c + 1) * P, :])
                bq_pool = ctx.enter_context(tc.tile_pool(name="bq", bufs=1))
                bqk_sb = bq_pool.tile([P, 16], f32, tag="bqk")
                nc.sync.dma_start(bqk_sb[:], bqk)
                bvp_sb = bq_pool.tile([P, VPW], f32, tag="bvp")
                nc.sync.dma_start(bvp_sb[:], bvp)
                mpool = ctx.enter_context(tc.tile_pool(name="masks", bufs=4))
                masks_sb = [mpool.tile([P, 1024], bf16, tag="mask", name=f"mask{i}")
                            for i in range(4)]
                for i in range(4):
                    nc.sync.dma_start(masks_sb[i][:], masks[i])

                # ---- x^T: transpose the whole batch up front -----------
                xT = [[None] * 8 for _ in range(4)]   # [ts][c] -> [128, 512]
                for ts in range(4):
                    for c in range(8):
                        tp = sh_p.tile([P, 512], bf16, tag="shp")
                        for tt in range(4):
                            nc.tensor.transpose(tp[:, tt * P:(tt + 1) * P],
                                                xrows[ts * 4 + tt][:, c * P:(c + 1) * P],
                                                ident_sb[:])
                        xc = xT_pool.tile([P, 512], bf16, tag="xT",
                                          name=f"xT{ts}_{c}")
                        nc.vector.tensor_copy(xc[:], tp[:])
                        xT[ts][c] = xc

                # ---- qkv emission units (software pipelining) ----------
                # Each unit emits one PSUM accumulation (8 matmuls + evict).
                v_sb = [[None] * NT for _ in range(4)]
                kt_tiles = {}
                qt_tiles = {}

                def unit_v(g, s):
                    def emit():
                        n0 = 260 * g
                        ts, tt = s // 4, s % 4
                        acc = sh_p.tile([P, 512], f32, tag="shp")
                        for c in range(8):
                            nc.tensor.matmul(acc[:, 0:260],
                                             xT[ts][c][:, tt * P:(tt + 1) * P],
                                             wvp_sb[c][:, n0:n0 + 260],
                                             start=(c == 0), stop=(c == 7))
                        vt = vs_pool.tile([P, 260], bf16, tag="vs",
                                          name=f"v{g}_{s}")
                        nc.vector.tensor_add(vt[:], acc[:, 0:260],
                                             bvp_sb[:, n0:n0 + 260])
                        v_sb[g][s] = vt
                    return emit

                def unit_k(p, ts):
                    def emit():
                        if p not in kt_tiles:
                            kt_tiles[p] = kt_pool.tile([P, T], bf16, tag="kt",
                                                       name=f"kt{p}")
                        kt = kt_tiles[p]
                        acc = sh_p.tile([P, 512], f32, tag="shp")
                        for c in range(8):
                            nc.tensor.matmul(acc[:],
                                             wqk_sb[c][:, (8 + p) * P:(9 + p) * P],
                                             xT[ts][c][:], start=(c == 0), stop=(c == 7))
                        nc.vector.tensor_scalar_add(kt[:, ts * 512:(ts + 1) * 512],
                                                    acc[:], bqk_sb[:, 8 + p:9 + p])
                    return emit

                def unit_q(p, qi):
                    def emit():
                        if p not in qt_tiles:
                            qt_tiles[p] = qt_pool.tile([P, 1024], bf16, tag="qt",
                                                       name=f"qt{p}")
                        qt = qt_tiles[p]
                        ts = (0, 2)[qi]
                        acc = sh_p.tile([P, 512], f32, tag="shp")
                        for c in range(8):
                            nc.tensor.matmul(acc[:],
                                             wqk_sb[c][:, p * P:(p + 1) * P],
                                             xT[ts][c][:], start=(c == 0), stop=(c == 7))
                        nc.vector.tensor_scalar_add(qt[:, qi * 512:(qi + 1) * 512],
                                                    acc[:], bqk_sb[:, p:p + 1])
                    return emit

                def qkv_units(p):
                    # K/Q first (next pack's attention gates on them), V' last
                    # (fills the attention-tail PE gap)
                    units = [unit_k(p, ts) for ts in range(4)]
                    units += [unit_q(p, qi) for qi in range(2)]
                    if p % 2 == 0:
                        units += [unit_v(p // 2, s) for s in range(NT)]
                    return units

                def norm_units(p, sums):
                    # per-pack normalize: batched reciprocal + bcast-mul per head
                    units = []
                    recb = sm_pool.tile([P, 512], bf16, tag="recb",
                                        name=f"recb{p}")

                    def u_recip():
                        with nc.allow_low_precision(reason="softmax recip"):
                            nc.vector.reciprocal(recb[:], sums[:])
                    units.append(u_recip)
                    for qb in range(2):
                        for hh in range(2):
                            def u_norm(qb=qb, hh=hh):
                                qsl = slice(qb * 512, qb * 512 + 512)
                                i = qb * 2 + hh
                                rcst = sm_pool.tile([1, 512], bf16, tag="rcst")
                                nc.vector.tensor_copy(rcst[:],
                                                      recb[32 * i:32 * i + 1, :])
                                bcp = sh_p.tile([64, 512], f32, tag="shp")
                                nc.tensor.matmul(bcp[:], ones_sb[:], rcst[:],
                                                 start=True, stop=True)
                                nc.vector.tensor_mul(
                                    yT[p][hh * 64:(hh + 1) * 64, qsl],
                                    yT[p][hh * 64:(hh + 1) * 64, qsl], bcp[:])
                            units.append(u_norm)
                    return units

                # ---- main pipeline over head-packs ---------------------
                for u in qkv_units(0):      # prologue
                    u()

                pend_norm = []
                for p in range(8):
                    pend = qkv_units(p + 1) if p < 8 - 1 else []
                    pend = pend[:6] + pend_norm + pend[6:]
                    total_u, emitted, si = len(pend), 0, 0
                    kt, qt = kt_tiles[p], qt_tiles[p]
                    g, off = p // 2, (p % 2) * 130
                    sums = sm_pool.tile([P, 512], f32, tag="sums")
                    for qb in range(2):
                        nk = NK0 if qb == 0 else NK1
                        qsl = slice(qb * 512, qb * 512 + 512)
                        y1 = yp_p.tile([HS + 1, 512], f32, tag="yp")
                        y2 = yp_p.tile([HS + 1, 512], f32, tag="yp")
                        for s in range(nk):
                            ksl = slice(s * P, (s + 1) * P)
                            span = span_p.tile([P, 1024], f32, tag="span")
                            nc.tensor.matmul(span[:, 0:512], kt[0:64, ksl],
                                             qt[0:64, qsl], start=True, stop=True)
                            nc.tensor.matmul(span[:, 512:1024], kt[64:128, ksl],
                                             qt[64:128, qsl], start=True, stop=True)
                            pt = pt_pool.tile([P, 1024], bf16, tag="pt")
                            bcol = biasc_sb[:, qb * 16 + s:qb * 16 + s + 1]
                            nc.scalar.activation(pt[:], span[:], AFT.Exp,
                                                 bias=bcol, scale=0.125)
                            mi = -1
                            if qb == 0 and s < 4:
                                mi = s
                            elif qb == 1 and 8 <= s < 12:
                                mi = s - 8
                            if mi >= 0:
                                nc.vector.tensor_mul(pt[:], pt[:], masks_sb[mi][:])
                            nc.tensor.matmul(y1[:], v_sb[g][s][:, off:off + 65],
                                             pt[:, 0:512],
                                             start=(s == 0), stop=(s == nk - 1))
                            nc.tensor.matmul(y2[:], v_sb[g][s][:, off + 65:off + 130],
                                             pt[:, 512:1024],
                                             start=(s == 0), stop=(s == nk - 1))
                            # spread next pack's qkv accumulations across this
                            # pack's attention slots (keeps PE fed while ACT
                            # drains the exp backlog)
                            si += 1
                            want = total_u * si // (NK0 + NK1)
                            while emitted < want:
                                pend.pop(0)()
                                emitted += 1
                        # stash raw y; collect denominators at aligned partitions
                        for hh, yy in ((0, y1), (1, y2)):
                            i = qb * 2 + hh
                            nc.vector.tensor_copy(sums[32 * i:32 * i + 1, :],
                                                  yy[64:65, :])
                            nc.vector.tensor_copy(
                                yT[p][hh * 64:(hh + 1) * 64, qsl], yy[0:64, :])

                    # normalize(p) runs interleaved into the next pack's slots
                    pend_norm = norm_units(p, sums)
                for u in pend_norm:
                    u()

            # ---------------- output projection --------------------------
            with ExitStack() as ctx:
                wp_pool = ctx.enter_context(tc.tile_pool(name="wpj", bufs=8))
                wpj_sb = [wp_pool.tile([P, C], bf16, tag="wpj", name=f"wpj{i}")
                          for i in range(8)]
                for c in range(8):
                    nc.sync.dma_start(wpj_sb[c][:], wpj[c * P:(c + 1) * P, :])
                bp_pool = ctx.enter_context(tc.tile_pool(name="bpj", bufs=1))
                bpj_sb = bp_pool.tile([P, C], f32, tag="bpj")
                nc.sync.dma_start(bpj_sb[:], bpj)

                pj_p = ctx.enter_context(tc.tile_pool(name="pj", bufs=4, space="PSUM"))
                ost = ctx.enter_context(tc.tile_pool(name="ost", bufs=3))
                for tt in range(8):
                    ot = ost.tile([P, C], f32, tag="ost")
                    for co in range(2):
                        acc = pj_p.tile([P, 512], f32, tag="pj")
                        for c in range(8):
                            nc.tensor.matmul(acc[:], yT[c][:, tt * P:(tt + 1) * P],
                                             wpj_sb[c][:, co * 512:(co + 1) * 512],
                                             start=(c == 0), stop=(c == 7))
                        nc.vector.tensor_add(ot[:, co * 512:(co + 1) * 512], acc[:],
                                             bpj_sb[:, co * 512:(co + 1) * 512])
                    nc.sync.dma_start(outd[tt * P:(tt + 1) * P, :], ot[:])

    nc.compile()
    return nc


_NC_CACHE = None


def _get_program():
    global _NC_CACHE
    if _NC_CACHE is None:
        _NC_CACHE = _build_program()
    return _NC_CACHE


def _host_inputs(x, W_attn, b_attn, W_proj, b_proj):
    """Build the 8 per-core input maps."""
    import ml_dtypes
    bf = ml_dtypes.bfloat16
    x = np.asarray(x, dtype=np.float32)
    W_attn = np.asarray(W_attn, dtype=np.float32)
    b_attn = np.asarray(b_attn, dtype=np.float32)
    W_proj = np.asarray(W_proj, dtype=np.float32)
    b_proj = np.asarray(b_proj, dtype=np.float32)

    wqk = np.ascontiguousarray(W_attn[:, :2 * C]).astype(bf)
    bqk = np.empty((P, 16), np.float32)
    for dt in range(16):
        bqk[:, dt] = b_attn[dt * P:(dt + 1) * P]
    # V' weights: per head 64 V columns + one zero column (ones come via bias)
    wvp = np.zeros((C, VPW), np.float32)
    bvp_row = np.zeros(VPW, np.float32)
    for h in range(NH):
        wvp[:, h * 65:h * 65 + 64] = W_attn[:, 2 * C + h * HS:2 * C + (h + 1) * HS]
        bvp_row[h * 65:h * 65 + 64] = b_attn[2 * C + h * HS:2 * C + (h + 1) * HS]
        bvp_row[h * 65 + 64] = 1.0
    wvp = wvp.astype(bf)
    bvp = np.tile(bvp_row, (P, 1))
    bpj = np.tile(b_proj, (P, 1))
    wpj = W_proj.astype(bf)

    # universal diagonal masks: mask_i[k, q] = 1 if 128*i + k <= q (dup for 2 heads)
    msk = np.zeros((4, P, 1024), np.float32)
    kk = np.arange(P)[:, None]
    qq = np.arange(512)[None, :]
    for i in range(4):
        m = (P * i + kk <= qq).astype(np.float32)
        msk[i, :, 0:512] = m
        msk[i, :, 512:1024] = m
    msk = msk.astype(bf)

    identm = np.eye(P, dtype=np.float32).astype(bf)
    ones64 = np.ones((1, 64), np.float32).astype(bf)

    in_maps = []
    for core in range(NCORES):
        b, xh = core // 2, core % 2
        order = TILE_ORDER[xh]
        tok = np.concatenate([np.arange(t * P, (t + 1) * P) for t in order])
        xc = np.ascontiguousarray(x[b][tok]).astype(bf)
        # exp bias columns [128, 32]: col = qb*16 + slot; -30000 kills acausal slots
        bc = np.zeros((P, 32), np.float32)
        if xh == 0:
            bc[:, 4:8] = -30000.0       # qb0 slots 4-7 dead (block 0)
        else:
            bc[:, 16 + 12:16 + 16] = -30000.0   # qb1 slots 12-15 dead (block 2)
        in_maps.append({
            "x": xc, "wqk": wqk, "bqk": bqk, "wvp": wvp, "bvp": bvp,
            "wproj": wpj, "bproj": bpj, "masks": msk, "biasc": bc,
            "ident": identm, "ones64": ones64,
        })
    return in_maps


def run(inputs, trace=False, tmpdir=None):
    from concourse.bass_utils import run_bass_kernel_spmd
    nc = _get_program()
    in_maps = _host_inputs(**inputs)
    res = run_bass_kernel_spmd(nc, in_maps, core_ids=list(range(NCORES)),
                               trace=trace, tmpdir=tmpdir)
    out = np.empty((B, T, C), np.float32)
    for core in range(NCORES):
        b, xh = core // 2, core % 2
        o = res.results[core]["out"]
        blk0, blk1 = (0, 3) if xh == 0 else (1, 2)
        out[b, blk0 * 512:(blk0 + 1) * 512] = o[0:512]
        out[b, blk1 * 512:(blk1 + 1) * 512] = o[512:1024]
    return out, res


def kernel(x, W_attn, b_attn, W_proj, b_proj):
    out, _ = run(dict(x=x, W_attn=W_attn, b_attn=b_attn,
                      W_proj=W_proj, b_proj=b_proj))
    return out
